# revision 7
# baseline (speedup 1.0000x reference)
"""DMPNN (NNConv/edge-network message passing) Trainium2 kernel, 8-core SPMD.

Sharding: edges are assigned to cores by dst-node range (512 nodes/core), so
scatter-mean partial sums are core-local; per layer one tiny AllReduce (BN
stats) + one AllGather (updated node features) cross the cores.

Device algorithm per layer, per 128-edge tile:
  e_vecT = relu(e1_w.T @ edge_attrT)                     (PE + DVE)
  W_e[h, e, o] = sum_k W2P[k, o*128+h] e_vecT[k, e]      (PE, 128 LDW+MM pairs,
                                                          evicted PSUM->SBUF bf16)
  msg_T[o, e] = h_srcT[:,e].T @ (B2 + W_e[:, e, :])       (PE, per-edge matvec)
  agg_u = S_t.T @ msg  (one-hot with 1/deg baked in)      (PE)  -> row scatter
Then: out = aggT + root_w.T @ hT_slice; BN via global stats AllReduce;
h += relu(BN(out)); AllGather h slices.
"""

import numpy as np
import ml_dtypes

import concourse.bass as bass
import concourse.tile as tile
import concourse.mybir as mybir
from concourse import bacc
from concourse.bass import IndirectOffsetOnAxis
from concourse.bass_utils import run_bass_kernel_spmd

BF16 = ml_dtypes.bfloat16

N, E, F_NODE, F_EDGE, H, L, G = 4096, 12288, 64, 16, 128, 4, 256
NC = 8
NS = N // NC          # nodes per core
P = 128
BN_EPS = 1e-5
AXF = mybir.ActivationFunctionType
ALU = mybir.AluOpType


# ----------------------------------------------------------------------------
# Host preprocessing
# ----------------------------------------------------------------------------

def _preprocess(edge_index, edge_attr):
    src = np.asarray(edge_index[0], dtype=np.int64)
    dst = np.asarray(edge_index[1], dtype=np.int64)
    edge_attr = np.asarray(edge_attr, dtype=np.float32)
    deg = np.bincount(dst, minlength=N).astype(np.float32)
    inv_deg = np.where(deg > 0, 1.0 / np.maximum(deg, 1.0), 0.0).astype(np.float32)

    core_of = dst // NS
    packed = []
    EP = 0
    for c in range(NC):
        idx = np.nonzero(core_of == c)[0]
        idx = idx[np.argsort(dst[idx], kind="stable")]
        d = dst[idx]
        # pack per-dst runs into 128-edge tiles; a run never crosses a tile
        slots = []
        fill = 0
        i = 0
        while i < len(idx):
            j = i
            while j < len(idx) and d[j] == d[i]:
                j += 1
            k = j - i
            if fill + k > P:
                slots.extend([-1] * (P - fill))
                fill = 0
            slots.extend(idx[i:j].tolist())
            fill = (fill + k) % P
            i = j
        if fill:
            slots.extend([-1] * (P - fill))
        packed.append(np.array(slots, dtype=np.int64))
        EP = max(EP, len(slots))
    EP = ((EP + P - 1) // P) * P
    T = EP // P

    cores = []
    for c in range(NC):
        slots = np.concatenate(
            [packed[c], -np.ones(EP - len(packed[c]), dtype=np.int64)]
        )
        valid = slots >= 0
        sl = np.maximum(slots, 0)
        e_src = np.where(valid, src[sl], 0).astype(np.int32)
        e_dst = np.where(valid, dst[sl], -1)
        ea = np.where(valid[:, None], edge_attr[sl], 0.0).astype(np.float32)
        S = np.zeros((T, P, P), dtype=np.float32)
        rows_u = np.full((T, P), NS, dtype=np.int32)  # NS = dummy row
        for t in range(T):
            ds = e_dst[t * P:(t + 1) * P]
            umap = {}
            for e in range(P):
                n_ = ds[e]
                if n_ < 0:
                    continue
                if n_ not in umap:
                    umap[n_] = len(umap)
                    rows_u[t, umap[n_]] = n_ - c * NS
                S[t, e, umap[n_]] = inv_deg[n_]
        # edge_attrT augmented with ones row: [17, EP]
        eaT = np.concatenate([ea.T, np.ones((1, EP), np.float32)], 0)
        cores.append(dict(eaT=eaT, srcg=e_src, rows_u=rows_u, S=S))
    return cores, EP, T


# ----------------------------------------------------------------------------
# Device program
# ----------------------------------------------------------------------------

def _build(EP, T):
    f32 = mybir.dt.float32
    bf16 = mybir.dt.bfloat16
    i32 = mybir.dt.int32
    nc = bacc.Bacc("TRN2", target_bir_lowering=False, debug=False, num_devices=NC)

    def din(name, shape, dt=bf16):
        return nc.dram_tensor(name, shape, dt, kind="ExternalInput")

    ea_d = din("ea", [17, EP])
    e1w_d = din("e1w", [L, 17, H])
    w2p_d = din("w2p", [L, H, H * H])
    b2_d = din("b2", [L, H, H])
    rw_d = din("rw", [L, H, H])
    bng_d = din("bng", [L, H, 1], f32)
    bnb_d = din("bnb", [L, H, 1], f32)
    xa_d = din("xa", [65, N])
    xs_d = din("xs", [65, NS])
    nw_d = din("nw", [65, H])
    srcg_d = din("srcg", [P, T], i32)
    rowu_d = din("rowu", [P, T], i32)
    smat_d = din("smat", [P, T * P], f32)
    pmat_d = din("pmat", [P, 32 * G])
    hw1_d = din("hw1", [H, H])
    hb1_d = din("hb1", [H, 1], f32)
    hw2_d = din("hw2", [H, 1])
    hb2_d = din("hb2", [1, 1], f32)
    idf_d = din("idf", [P, P], f32)
    idb_d = din("idb", [P, P])
    y_d = nc.dram_tensor("y", [1, G], f32, kind="ExternalOutput")

    groups = [list(range(NC))]

    with tile.TileContext(nc) as tc:
        with tc.tile_pool(name="const", bufs=1) as const, \
             tc.tile_pool(name="persist", bufs=1) as persist, \
             tc.tile_pool(name="spool", bufs=3) as spool, \
             tc.tile_pool(name="gpool", bufs=4) as gpool, \
             tc.tile_pool(name="stat", bufs=2) as statp, \
             tc.tile_pool(name="psw", bufs=2, space="PSUM") as psum_we, \
             tc.tile_pool(name="psm", bufs=2, space="PSUM") as psum_msg, \
             tc.tile_pool(name="psx", bufs=3, space="PSUM") as psum_m, \
             tc.tile_pool(name="psr", bufs=1, space="PSUM") as psum_root, \
             tc.tile_pool(name="dramp", bufs=2, space="DRAM") as dramp:

            # ---- persistent constants in SBUF ----
            ea_sb = const.tile([17, EP], bf16)
            nc.sync.dma_start(ea_sb[:], ea_d[:])
            nw_sb = const.tile([65, H], bf16)
            nc.sync.dma_start(nw_sb[:], nw_d[:])
            xs_sb = const.tile([65, NS], bf16)
            nc.sync.dma_start(xs_sb[:], xs_d[:])
            idf_sb = const.tile([P, P], f32)
            nc.sync.dma_start(idf_sb[:], idf_d[:])
            idb_sb = const.tile([P, P], bf16)
            nc.sync.dma_start(idb_sb[:], idb_d[:])
            srcg_sb = const.tile([P, T], i32)
            nc.sync.dma_start(srcg_sb[:], srcg_d[:])
            rowu_sb = const.tile([P, T], i32)
            nc.sync.dma_start(rowu_sb[:], rowu_d[:])
            smat_sb = const.tile([P, T * P], f32)
            nc.sync.dma_start(smat_sb[:], smat_d[:])
            e1w_sb = []
            b2_sb = []
            rw_sb = []
            bng_sb = []
            bnb_sb = []
            for l in range(L):
                e1w_l = const.tile([17, H], bf16, name=f"e1w_{l}")
                nc.sync.dma_start(e1w_l[:], e1w_d[l])
                e1w_sb.append(e1w_l)
                b2_l = const.tile([H, H], bf16, name=f"b2_{l}")
                nc.sync.dma_start(b2_l[:], b2_d[l])
                b2_sb.append(b2_l)
                rw_l = const.tile([H, H], bf16, name=f"rw_{l}")
                nc.sync.dma_start(rw_l[:], rw_d[l])
                rw_sb.append(rw_l)
                bng_l = const.tile([H, 1], f32, name=f"bng_{l}")
                nc.sync.dma_start(bng_l[:], bng_d[l])
                bng_sb.append(bng_l)
                bnb_l = const.tile([H, 1], f32, name=f"bnb_{l}")
                nc.sync.dma_start(bnb_l[:], bnb_d[l])
                bnb_sb.append(bnb_l)
            ztile = const.tile([P, H], f32)
            nc.vector.memset(ztile[:], 0.0)
            eps_sb = const.tile([H, 1], f32)
            nc.vector.memset(eps_sb[:], BN_EPS)

            hT = persist.tile([H, NS], f32)  # own slice, [h, n] fp32

            # ---- node encoder ----
            h0 = dramp.tile([N, H], bf16, bufs=1)
            with tc.tile_pool(name="encp", bufs=2) as encp:
                xa_sb = encp.tile([65, N], bf16, bufs=1)
                nc.sync.dma_start(xa_sb[:], xa_d[:])
                for i in range(N // P):
                    ps = psum_m.tile([P, H], f32, name="enc_ps", tag="m")
                    nc.tensor.matmul(ps[:], xa_sb[:, i * P:(i + 1) * P], nw_sb[:],
                                     start=True, stop=True)
                    hrow = encp.tile([P, H], bf16, name="enc_row")
                    eng = nc.vector if i % 2 == 0 else nc.scalar
                    if i % 2 == 0:
                        eng.tensor_copy(hrow[:], ps[:])
                    else:
                        eng.copy(hrow[:], ps[:])
                    nc.sync.dma_start(h0[i * P:(i + 1) * P, :], hrow[:])
                # own slice, transposed fp32
                for j in range(NS // P):
                    ps = psum_m.tile([P, H], f32, name="enc_ps2", tag="m")
                    nc.tensor.matmul(ps[:], xs_sb[:, j * P:(j + 1) * P], nw_sb[:],
                                     start=True, stop=True)
                    tmp = encp.tile([P, H], f32, name="enc_tmp")
                    nc.vector.tensor_copy(tmp[:], ps[:])
                    ps2 = psum_m.tile([P, P], f32, name="enc_ps3", tag="m")
                    nc.tensor.transpose(ps2[:], tmp[:], idf_sb[:])
                    nc.scalar.copy(hT[:, j * P:(j + 1) * P], ps2[:])

            h_rows = h0

            # ---- layers ----
            with tc.tile_pool(name="w2pool", bufs=2) as w2pool, \
                 tc.tile_pool(name="wepool", bufs=2) as wepool, \
                 tc.tile_pool(name="evpool", bufs=2) as evpool:
                for l in range(L):
                    w2_sb = w2pool.tile([H, H * H], bf16, name="w2")
                    nc.sync.dma_start(w2_sb[:], w2p_d[l])

                    # e_vecT = relu(e1wT @ eaT): [128k, EP]
                    evT = evpool.tile([H, EP], bf16, name="ev")
                    for c0 in range(0, EP, 512):
                        w = min(512, EP - c0)
                        ps = psum_we.tile([P, 512], f32, name="ev_ps", tag="w512")
                        nc.tensor.matmul(ps[:, :w], e1w_sb[l][:],
                                         ea_sb[:, c0:c0 + w], start=True, stop=True)
                        nc.vector.tensor_scalar_max(evT[:, c0:c0 + w], ps[:, :w], 0.0)

                    # root term: [128o, NS] = rw.T @ hT(bf16)
                    hTb = spool.tile([H, NS], bf16, name="hTb")
                    nc.vector.tensor_copy(hTb[:], hT[:])
                    ps_root = psum_root.tile([P, NS], f32, name="root_ps", tag="r")
                    nc.tensor.matmul(ps_root[:], rw_sb[l][:], hTb[:],
                                     start=True, stop=True)

                    # agg staging in DRAM, zeroed
                    aggd = dramp.tile([NS + 8, H], f32, name="aggd")
                    for j in range(NS // P):
                        nc.sync.dma_start(aggd[j * P:(j + 1) * P, :], ztile[:])

                    for t in range(T):
                        # gather h[src] rows -> [128e, 128h] bf16
                        hs = gpool.tile([P, H], bf16, name="hs")
                        nc.gpsimd.indirect_dma_start(
                            out=hs[:], out_offset=None, in_=h_rows[:],
                            in_offset=IndirectOffsetOnAxis(
                                ap=srcg_sb[:, t:t + 1], axis=0),
                        )
                        pst = psum_m.tile([P, P], bf16, name="hsT_ps", tag="m")
                        nc.tensor.transpose(pst[:], hs[:], idb_sb[:])
                        hsT = spool.tile([P, P], bf16, name="hsT")
                        nc.scalar.copy(hsT[:], pst[:])

                        # W_e generation: [h, e*128+o] bf16
                        we = wepool.tile([H, H * H], bf16, name="we")
                        we_v = we.rearrange("p (e o) -> p o e", o=H)
                        ev_t = evT[:, t * P:(t + 1) * P]
                        for og in range(32):
                            psw = psum_we.tile([P, 512], f32, name="we_ps", tag="w512")
                            for oi in range(4):
                                o = og * 4 + oi
                                nc.tensor.matmul(
                                    psw[:, oi * P:(oi + 1) * P],
                                    w2_sb[:, o * P:(o + 1) * P], ev_t,
                                    start=True, stop=True)
                            psw_v = psw.rearrange("p (o e) -> p o e", o=4)
                            if og % 2 == 0:
                                nc.vector.tensor_copy(
                                    we_v[:, og * 4:og * 4 + 4, :], psw_v[:])
                            else:
                                nc.scalar.copy(
                                    we_v[:, og * 4:og * 4 + 4, :], psw_v[:])

                        # per-edge matvec: msg_T[o, e]
                        psm = psum_msg.tile([P, P], f32, name="msg_ps")
                        nc.tensor.matmul(psm[:], b2_sb[l][:], hsT[:],
                                         start=True, stop=False,
                                         skip_group_check=True)
                        for e in range(P):
                            nc.tensor.matmul(
                                psm[:, e:e + 1], we[:, e * H:(e + 1) * H],
                                hsT[:, e:e + 1], start=False, stop=(e == P - 1),
                                skip_group_check=True)
                        msgT = spool.tile([P, P], f32, name="msgT")
                        nc.vector.tensor_copy(msgT[:], psm[:])
                        psm2 = psum_m.tile([P, P], f32, name="msg_t_ps", tag="m")
                        nc.tensor.transpose(psm2[:], msgT[:], idf_sb[:])
                        msgE = spool.tile([P, P], f32, name="msgE")
                        nc.scalar.copy(msgE[:], psm2[:])

                        # scatter matmul + row scatter
                        psa = psum_m.tile([P, P], f32, name="agg_ps", tag="m")
                        nc.tensor.matmul(psa[:], smat_sb[:, t * P:(t + 1) * P],
                                         msgE[:], start=True, stop=True)
                        aggu = spool.tile([P, P], f32, name="aggu")
                        nc.vector.tensor_copy(aggu[:], psa[:])
                        nc.gpsimd.indirect_dma_start(
                            out=aggd[:], in_=aggu[:], in_offset=None,
                            out_offset=IndirectOffsetOnAxis(
                                ap=rowu_sb[:, t:t + 1], axis=0),
                        )

                    # aggT [128o, NS] via on-chip PE transposes of agg row tiles
                    aggT = spool.tile([H, NS], f32, name="aggT")
                    for j in range(NS // P):
                        agr = gpool.tile([P, H], f32, name="agr")
                        nc.sync.dma_start(agr[:], aggd[j * P:(j + 1) * P, :])
                        psq = psum_m.tile([P, P], f32, name="agr_ps", tag="m")
                        nc.tensor.transpose(psq[:], agr[:], idf_sb[:])
                        nc.scalar.copy(aggT[:, j * P:(j + 1) * P], psq[:])

                    outT = spool.tile([H, NS], f32, name="outT")
                    nc.vector.tensor_add(outT[:], aggT[:], ps_root[:])

                    # BN stats: global sum & sum-of-squares over nodes
                    stats = statp.tile([H, 2], f32, name="stats")
                    nc.vector.tensor_reduce(stats[:, 0:1], outT[:],
                                            axis=mybir.AxisListType.X, op=ALU.add)
                    trash = spool.tile([H, NS], f32, name="trash")
                    nc.scalar.activation(trash[:], outT[:], AXF.Square,
                                         accum_out=stats[:, 1:2])
                    st_in = dramp.tile([H, 2], f32, name="st_in")
                    nc.sync.dma_start(st_in[:], stats[:])
                    st_out = dramp.tile([H, 2], f32, name="st_out",
                                        addr_space="Shared")
                    nc.gpsimd.collective_compute(
                        "AllReduce", ALU.add, replica_groups=groups,
                        ins=[st_in.opt()], outs=[st_out.opt()])
                    stats2 = statp.tile([H, 2], f32, name="stats2")
                    nc.sync.dma_start(stats2[:], st_out[:])

                    mu = statp.tile([H, 1], f32, name="mu")
                    nc.scalar.mul(mu[:], stats2[:, 0:1], 1.0 / N)
                    ex2 = statp.tile([H, 1], f32, name="ex2")
                    nc.scalar.mul(ex2[:], stats2[:, 1:2], 1.0 / N)
                    musq = statp.tile([H, 1], f32, name="musq")
                    nc.vector.tensor_mul(musq[:], mu[:], mu[:])
                    var = statp.tile([H, 1], f32, name="var")
                    nc.vector.tensor_tensor(out=var[:], in0=ex2[:], in1=musq[:],
                                            op=ALU.subtract)
                    std = statp.tile([H, 1], f32, name="std")
                    nc.scalar.activation(std[:], var[:], AXF.Sqrt,
                                         bias=eps_sb[:, 0:1])
                    rstd = statp.tile([H, 1], f32, name="rstd")
                    nc.vector.reciprocal(rstd[:], std[:])
                    scal = statp.tile([H, 1], f32, name="scal")
                    nc.vector.tensor_mul(scal[:], rstd[:], bng_sb[l][:])
                    mscal = statp.tile([H, 1], f32, name="mscal")
                    nc.vector.tensor_mul(mscal[:], mu[:], scal[:])
                    shift = statp.tile([H, 1], f32, name="shift")
                    nc.vector.tensor_tensor(out=shift[:], in0=bnb_sb[l][:],
                                            in1=mscal[:], op=ALU.subtract)

                    relu_o = spool.tile([H, NS], f32, name="relu_o")
                    nc.scalar.activation(relu_o[:], outT[:], AXF.Relu,
                                         bias=shift[:, 0:1], scale=scal[:, 0:1])
                    nc.vector.tensor_add(hT[:], hT[:], relu_o[:])

                    # write updated slice (rows, bf16) + AllGather
                    hsl = dramp.tile([NS, H], bf16, name="hsl")
                    for j in range(NS // P):
                        pst = psum_m.tile([P, P], f32, name="hup_ps", tag="m")
                        nc.tensor.transpose(pst[:], hT[:, j * P:(j + 1) * P],
                                            idf_sb[:])
                        hrow = spool.tile([P, H], bf16, name="hup_row")
                        nc.scalar.copy(hrow[:], pst[:])
                        nc.sync.dma_start(hsl[j * P:(j + 1) * P, :], hrow[:])
                    # NOTE: Local (not Shared) — indirect-DMA gather reads this
                    # tensor, and gathering from the Shared window hangs HW.
                    hfull = dramp.tile([N, H], bf16, name="hfull")
                    nc.gpsimd.collective_compute(
                        "AllGather", ALU.bypass, replica_groups=groups,
                        ins=[hsl.opt()], outs=[hfull.opt()])
                    h_rows = hfull

            # ---- head (all cores redundantly) ----
            with tc.tile_pool(name="headp", bufs=2) as headp:
                pmat_sb = headp.tile([P, 32 * G], bf16, bufs=1)
                nc.sync.dma_start(pmat_sb[:], pmat_d[:])
                hw1_sb = headp.tile([H, H], bf16, bufs=1)
                nc.sync.dma_start(hw1_sb[:], hw1_d[:])
                hb1_sb = headp.tile([H, 1], f32, bufs=1)
                nc.sync.dma_start(hb1_sb[:], hb1_d[:])
                hw2_sb = headp.tile([H, 1], bf16, bufs=1)
                nc.sync.dma_start(hw2_sb[:], hw2_d[:])
                hb2_sb = headp.tile([1, 1], f32, bufs=1)
                nc.sync.dma_start(hb2_sb[:], hb2_d[:])

                ps_pool = psum_root.tile([H, G], f32, name="pool_ps", tag="r")
                for i in range(N // P):
                    hrt = headp.tile([P, H], bf16, name="hd_rows")
                    nc.sync.dma_start(hrt[:], h_rows[i * P:(i + 1) * P, :])
                    nc.tensor.matmul(ps_pool[:], hrt[:],
                                     pmat_sb[:, i * G:(i + 1) * G],
                                     start=(i == 0), stop=(i == N // P - 1))
                pooledT = headp.tile([H, G], bf16, name="pooledT")
                nc.vector.tensor_copy(pooledT[:], ps_pool[:])
                ps_z = psum_m.tile([H, G], f32, name="z_ps", tag="m")
                nc.tensor.matmul(ps_z[:], hw1_sb[:], pooledT[:],
                                 start=True, stop=True)
                z = headp.tile([H, G], bf16, name="z")
                nc.scalar.activation(z[:], ps_z[:], AXF.Relu, bias=hb1_sb[:, 0:1])
                ps_y = psum_m.tile([1, G], f32, name="y_ps", tag="m")
                nc.tensor.matmul(ps_y[:], hw2_sb[:], z[:], start=True, stop=True)
                ysb = headp.tile([1, G], f32, name="ysb")
                nc.vector.tensor_scalar_add(ysb[:], ps_y[:], hb2_sb[0:1, 0:1])
                nc.sync.dma_start(y_d[:], ysb[:])

    nc.compile()
    return nc


# ----------------------------------------------------------------------------
# Entry point
# ----------------------------------------------------------------------------

def kernel(**inputs):
    inp = {k: np.asarray(v) for k, v in inputs.items()}
    cores, EP, T = _preprocess(inp["edge_index"], inp["edge_attr"])

    bf = lambda a: np.ascontiguousarray(np.asarray(a, np.float32)).astype(BF16)
    f32 = lambda a: np.ascontiguousarray(np.asarray(a, np.float32))

    # shared (replicated) tensors
    e1w = np.concatenate(
        [np.asarray(inp["e1_w"], np.float32),
         np.asarray(inp["e1_b"], np.float32)[:, None, :]], axis=1)  # [L,17,128]
    # W2P[l][k, o*128+h] = e2_w[l][k, h*128+o]
    w2p = np.asarray(inp["e2_w"], np.float32).reshape(L, H, H, H) \
        .transpose(0, 1, 3, 2).reshape(L, H, H * H)
    b2 = np.asarray(inp["e2_b"], np.float32).reshape(L, H, H)
    xa = np.concatenate([np.asarray(inp["x"], np.float32).T,
                         np.ones((1, N), np.float32)], 0)  # [65, N]
    nw = np.concatenate([np.asarray(inp["node_w"], np.float32),
                         np.asarray(inp["node_b"], np.float32)[None, :]], 0)

    batch = np.asarray(inp["batch"], np.int64)
    cnt = np.bincount(batch, minlength=G).astype(np.float32)
    Pm = np.zeros((N, G), np.float32)
    Pm[np.arange(N), batch] = 1.0 / np.maximum(cnt, 1.0)[batch]
    pmat = np.zeros((P, 32 * G), np.float32)
    for i in range(32):
        pmat[:, i * G:(i + 1) * G] = Pm[i * P:(i + 1) * P]

    shared = dict(
        e1w=bf(e1w), w2p=bf(w2p), b2=bf(b2),
        rw=bf(inp["root_w"]),
        bng=f32(inp["bn_g"])[:, :, None], bnb=f32(inp["bn_b"])[:, :, None],
        xa=bf(xa), nw=bf(nw),
        pmat=bf(pmat), hw1=bf(inp["head_w1"]),
        hb1=f32(inp["head_b1"])[:, None], hw2=bf(inp["head_w2"]),
        hb2=f32(inp["head_b2"])[None, :],
        idf=np.eye(P, dtype=np.float32), idb=np.eye(P, dtype=np.float32).astype(BF16),
    )

    in_maps = []
    for c in range(NC):
        cd = cores[c]
        m = dict(shared)
        m["ea"] = bf(cd["eaT"])
        m["xs"] = bf(xa[:, c * NS:(c + 1) * NS])
        m["srcg"] = np.ascontiguousarray(cd["srcg"].reshape(T, P).T)
        m["rowu"] = np.ascontiguousarray(cd["rows_u"].T)
        m["smat"] = np.ascontiguousarray(
            cd["S"].transpose(1, 0, 2).reshape(P, T * P))
        in_maps.append(m)

    nc = _build(EP, T)
    import os
    trace = os.environ.get("KERNEL_TRACE", "0") == "1"
    res = run_bass_kernel_spmd(nc, in_maps, list(range(NC)), trace=trace)
    if trace and res.exec_time_ns is not None:
        print(f"HW exec time: {res.exec_time_ns} ns")
    y = np.asarray(res.results[0]["y"], np.float32).reshape(G)
    return y


# revision 10
# speedup vs baseline: 1.4172x; 1.4172x over previous
"""DMPNN (NNConv/edge-network message passing) Trainium2 kernel, 8-core SPMD.

Sharding: edges are assigned to cores by dst-node range (512 nodes/core), so
scatter-mean partial sums are core-local; per layer one tiny AllReduce (BN
stats) + one AllGather (updated node features) cross the cores.

Device algorithm per layer, per 128-edge tile:
  e_vecT = relu(e1_w.T @ edge_attrT)                     (PE + DVE)
  W_e[h, e, o] = sum_k W2P[k, o*128+h] e_vecT[k, e]      (PE, 128 LDW+MM pairs,
                                                          evicted PSUM->SBUF bf16)
  msg_T[o, e] = h_srcT[:,e].T @ (B2 + W_e[:, e, :])       (PE, per-edge matvec)
  agg_u = S_t.T @ msg  (one-hot with 1/deg baked in)      (PE)  -> row scatter
Then: out = aggT + root_w.T @ hT_slice; BN via global stats AllReduce;
h += relu(BN(out)); AllGather h slices.
"""

import numpy as np
import ml_dtypes

import concourse.bass as bass
import concourse.tile as tile
import concourse.mybir as mybir
from concourse import bacc
from concourse.bass import IndirectOffsetOnAxis
from concourse.bass_utils import run_bass_kernel_spmd

BF16 = ml_dtypes.bfloat16

N, E, F_NODE, F_EDGE, H, L, G = 4096, 12288, 64, 16, 128, 4, 256
NC = 8
NS = N // NC          # nodes per core
P = 128
BN_EPS = 1e-5
AXF = mybir.ActivationFunctionType
ALU = mybir.AluOpType


# ----------------------------------------------------------------------------
# Host preprocessing
# ----------------------------------------------------------------------------

def _preprocess(edge_index, edge_attr):
    src = np.asarray(edge_index[0], dtype=np.int64)
    dst = np.asarray(edge_index[1], dtype=np.int64)
    edge_attr = np.asarray(edge_attr, dtype=np.float32)
    deg = np.bincount(dst, minlength=N).astype(np.float32)
    inv_deg = np.where(deg > 0, 1.0 / np.maximum(deg, 1.0), 0.0).astype(np.float32)

    core_of = dst // NS
    packed = []
    EP = 0
    for c in range(NC):
        idx = np.nonzero(core_of == c)[0]
        idx = idx[np.argsort(dst[idx], kind="stable")]
        d = dst[idx]
        # pack per-dst runs into 128-edge tiles; a run never crosses a tile
        slots = []
        fill = 0
        i = 0
        while i < len(idx):
            j = i
            while j < len(idx) and d[j] == d[i]:
                j += 1
            k = j - i
            if fill + k > P:
                slots.extend([-1] * (P - fill))
                fill = 0
            slots.extend(idx[i:j].tolist())
            fill = (fill + k) % P
            i = j
        if fill:
            slots.extend([-1] * (P - fill))
        packed.append(np.array(slots, dtype=np.int64))
        EP = max(EP, len(slots))
    EP = ((EP + P - 1) // P) * P
    T = EP // P

    cores = []
    for c in range(NC):
        slots = np.concatenate(
            [packed[c], -np.ones(EP - len(packed[c]), dtype=np.int64)]
        )
        valid = slots >= 0
        sl = np.maximum(slots, 0)
        e_src = np.where(valid, src[sl], 0).astype(np.int32)
        e_dst = np.where(valid, dst[sl], -1)
        ea = np.where(valid[:, None], edge_attr[sl], 0.0).astype(np.float32)
        S = np.zeros((T, P, P), dtype=np.float32)
        rows_u = np.full((T, P), NS, dtype=np.int32)  # NS = dummy row
        for t in range(T):
            ds = e_dst[t * P:(t + 1) * P]
            umap = {}
            for e in range(P):
                n_ = ds[e]
                if n_ < 0:
                    continue
                if n_ not in umap:
                    umap[n_] = len(umap)
                    rows_u[t, umap[n_]] = n_ - c * NS
                S[t, e, umap[n_]] = inv_deg[n_]
        # edge_attrT augmented with ones row: [17, EP]
        eaT = np.concatenate([ea.T, np.ones((1, EP), np.float32)], 0)
        cores.append(dict(eaT=eaT, srcg=e_src, rows_u=rows_u, S=S))
    return cores, EP, T


# ----------------------------------------------------------------------------
# Device program
# ----------------------------------------------------------------------------

def _build(EP, T):
    f32 = mybir.dt.float32
    bf16 = mybir.dt.bfloat16
    i32 = mybir.dt.int32
    nc = bacc.Bacc("TRN2", target_bir_lowering=False, debug=False, num_devices=NC)

    def din(name, shape, dt=bf16):
        return nc.dram_tensor(name, shape, dt, kind="ExternalInput")

    ea_d = din("ea", [17, EP])
    e1w_d = din("e1w", [L, 17, H])
    w2p_d = din("w2p", [L, H, H * H])
    b2_d = din("b2", [L, H, H])
    rw_d = din("rw", [L, H, H])
    bng_d = din("bng", [L, H, 1], f32)
    bnb_d = din("bnb", [L, H, 1], f32)
    xa_d = din("xa", [65, N])
    xs_d = din("xs", [65, NS])
    nw_d = din("nw", [65, H])
    srcg_d = din("srcg", [P, T], i32)
    rowu_d = din("rowu", [P, T], i32)
    smat_d = din("smat", [P, T * P], f32)
    pmat_d = din("pmat", [P, 32 * G])
    hw1_d = din("hw1", [H, H])
    hb1_d = din("hb1", [H, 1], f32)
    hw2_d = din("hw2", [H, 1])
    hb2_d = din("hb2", [1, 1], f32)
    idf_d = din("idf", [P, P], f32)
    idb_d = din("idb", [P, P])
    y_d = nc.dram_tensor("y", [1, G], f32, kind="ExternalOutput")

    groups = [list(range(NC))]

    with tile.TileContext(nc) as tc:
        with tc.tile_pool(name="const", bufs=1) as const, \
             tc.tile_pool(name="persist", bufs=1) as persist, \
             tc.tile_pool(name="spool", bufs=3) as spool, \
             tc.tile_pool(name="gpool", bufs=4) as gpool, \
             tc.tile_pool(name="stat", bufs=2) as statp, \
             tc.tile_pool(name="psw", bufs=2, space="PSUM") as psum_we, \
             tc.tile_pool(name="psm", bufs=2, space="PSUM") as psum_msg, \
             tc.tile_pool(name="psx", bufs=3, space="PSUM") as psum_m, \
             tc.tile_pool(name="psr", bufs=1, space="PSUM") as psum_root, \
             tc.tile_pool(name="dramp", bufs=2, space="DRAM") as dramp:

            # ---- persistent constants in SBUF ----
            ea_sb = const.tile([17, EP], bf16)
            nc.sync.dma_start(ea_sb[:], ea_d[:])
            nw_sb = const.tile([65, H], bf16)
            nc.sync.dma_start(nw_sb[:], nw_d[:])
            xs_sb = const.tile([65, NS], bf16)
            nc.sync.dma_start(xs_sb[:], xs_d[:])
            idf_sb = const.tile([P, P], f32)
            nc.sync.dma_start(idf_sb[:], idf_d[:])
            idb_sb = const.tile([P, P], bf16)
            nc.sync.dma_start(idb_sb[:], idb_d[:])
            srcg_sb = const.tile([P, T], i32)
            nc.sync.dma_start(srcg_sb[:], srcg_d[:])
            rowu_sb = const.tile([P, T], i32)
            nc.sync.dma_start(rowu_sb[:], rowu_d[:])
            smat_sb = const.tile([P, T * P], f32)
            nc.sync.dma_start(smat_sb[:], smat_d[:])
            e1w_sb = []
            b2_sb = []
            rw_sb = []
            bng_sb = []
            bnb_sb = []
            for l in range(L):
                e1w_l = const.tile([17, H], bf16, name=f"e1w_{l}")
                nc.sync.dma_start(e1w_l[:], e1w_d[l])
                e1w_sb.append(e1w_l)
                b2_l = const.tile([H, H], bf16, name=f"b2_{l}")
                nc.sync.dma_start(b2_l[:], b2_d[l])
                b2_sb.append(b2_l)
                rw_l = const.tile([H, H], bf16, name=f"rw_{l}")
                nc.sync.dma_start(rw_l[:], rw_d[l])
                rw_sb.append(rw_l)
                bng_l = const.tile([H, 1], f32, name=f"bng_{l}")
                nc.sync.dma_start(bng_l[:], bng_d[l])
                bng_sb.append(bng_l)
                bnb_l = const.tile([H, 1], f32, name=f"bnb_{l}")
                nc.sync.dma_start(bnb_l[:], bnb_d[l])
                bnb_sb.append(bnb_l)
            ztile = const.tile([P, H], f32)
            nc.vector.memset(ztile[:], 0.0)
            eps_sb = const.tile([H, 1], f32)
            nc.vector.memset(eps_sb[:], BN_EPS)

            hT = persist.tile([H, NS], f32)  # own slice, [h, n] fp32

            # ---- node encoder ----
            h0 = dramp.tile([N, H], bf16, bufs=1)
            with tc.tile_pool(name="encp", bufs=2) as encp:
                xa_sb = encp.tile([65, N], bf16, bufs=1)
                nc.sync.dma_start(xa_sb[:], xa_d[:])
                for i in range(N // P):
                    ps = psum_m.tile([P, H], f32, name="enc_ps", tag="m")
                    nc.tensor.matmul(ps[:], xa_sb[:, i * P:(i + 1) * P], nw_sb[:],
                                     start=True, stop=True)
                    hrow = encp.tile([P, H], bf16, name="enc_row")
                    eng = nc.vector if i % 2 == 0 else nc.scalar
                    if i % 2 == 0:
                        eng.tensor_copy(hrow[:], ps[:])
                    else:
                        eng.copy(hrow[:], ps[:])
                    nc.sync.dma_start(h0[i * P:(i + 1) * P, :], hrow[:])
                # own slice, transposed fp32
                for j in range(NS // P):
                    ps = psum_m.tile([P, H], f32, name="enc_ps2", tag="m")
                    nc.tensor.matmul(ps[:], xs_sb[:, j * P:(j + 1) * P], nw_sb[:],
                                     start=True, stop=True)
                    tmp = encp.tile([P, H], f32, name="enc_tmp")
                    nc.vector.tensor_copy(tmp[:], ps[:])
                    ps2 = psum_m.tile([P, P], f32, name="enc_ps3", tag="m")
                    nc.tensor.transpose(ps2[:], tmp[:], idf_sb[:])
                    nc.scalar.copy(hT[:, j * P:(j + 1) * P], ps2[:])

            h_rows = h0

            # ---- layers ----
            with tc.tile_pool(name="w2pool", bufs=2) as w2pool, \
                 tc.tile_pool(name="wepool", bufs=2) as wepool, \
                 tc.tile_pool(name="evpool", bufs=2) as evpool:
                for l in range(L):
                    w2_sb = w2pool.tile([H, H * H], bf16, name="w2")
                    nc.sync.dma_start(w2_sb[:], w2p_d[l])

                    # e_vecT = relu(e1wT @ eaT): [128k, EP]
                    evT = evpool.tile([H, EP], bf16, name="ev")
                    for c0 in range(0, EP, 512):
                        w = min(512, EP - c0)
                        ps = psum_we.tile([P, 512], f32, name="ev_ps", tag="w512")
                        nc.tensor.matmul(ps[:, :w], e1w_sb[l][:],
                                         ea_sb[:, c0:c0 + w], start=True, stop=True)
                        nc.vector.tensor_scalar_max(evT[:, c0:c0 + w], ps[:, :w], 0.0)

                    # root term: [128o, NS] = rw.T @ hT(bf16)
                    hTb = spool.tile([H, NS], bf16, name="hTb")
                    nc.vector.tensor_copy(hTb[:], hT[:])
                    ps_root = psum_root.tile([P, NS], f32, name="root_ps", tag="r")
                    nc.tensor.matmul(ps_root[:], rw_sb[l][:], hTb[:],
                                     start=True, stop=True)

                    # agg staging in DRAM, zeroed
                    aggd = dramp.tile([NS + 8, H], f32, name="aggd")
                    for j in range(NS // P):
                        nc.sync.dma_start(aggd[j * P:(j + 1) * P, :], ztile[:])

                    # edge groups of up to 2 tiles (256 edges) share one
                    # W_e buffer; o-outer generation amortizes the W2P LDW
                    # and makes PSUM->SBUF evictions fully contiguous.
                    for g0 in range(0, T, 2):
                        gt = min(2, T - g0)          # tiles in this group
                        ge = gt * P                  # edges in this group
                        ev_g = evT[:, g0 * P:g0 * P + ge]

                        # h_srcT for each tile in the group
                        hsTs = []
                        for ti in range(gt):
                            t = g0 + ti
                            hs = gpool.tile([P, H], bf16, name="hs")
                            nc.gpsimd.indirect_dma_start(
                                out=hs[:], out_offset=None, in_=h_rows[:],
                                in_offset=IndirectOffsetOnAxis(
                                    ap=srcg_sb[:, t:t + 1], axis=0),
                            )
                            pst = psum_m.tile([P, P], bf16, name="hsT_ps", tag="m")
                            nc.tensor.transpose(pst[:], hs[:], idb_sb[:])
                            hsT = spool.tile([P, P], bf16, name="hsT", bufs=4)
                            nc.scalar.copy(hsT[:], pst[:])
                            hsTs.append(hsT)

                        # W_e generation: we[h, o*ge + e] (o-outer, e-inner)
                        we = wepool.tile([H, H * 2 * P], bf16, name="we", bufs=1)
                        for og in range(H // 2):
                            psw = psum_we.tile([P, 512], f32, name="we_ps",
                                               tag="w512")
                            for oi in range(2):
                                o = og * 2 + oi
                                nc.tensor.matmul(
                                    psw[:, oi * ge:(oi + 1) * ge],
                                    w2_sb[:, o * P:(o + 1) * P], ev_g,
                                    start=True, stop=True)
                            if og % 2 == 0:
                                nc.vector.tensor_copy(
                                    we[:, og * 2 * ge:(og * 2 + 2) * ge],
                                    psw[:, :2 * ge])
                            else:
                                nc.scalar.copy(
                                    we[:, og * 2 * ge:(og * 2 + 2) * ge],
                                    psw[:, :2 * ge])
                        # we viewed as [h, o, e(ge)]
                        we_v = we[:, :H * ge].rearrange("p (o e) -> p o e", o=H)

                        for ti in range(gt):
                            t = g0 + ti
                            hsT = hsTs[ti]
                            # per-edge matvec: msg_T[o, e] (stationary =
                            # W_e[e] as [h, o]-strided view, rhs = h col)
                            psm = psum_msg.tile([P, P], f32, name="msg_ps")
                            nc.tensor.matmul(psm[:], b2_sb[l][:], hsT[:],
                                             start=True, stop=False,
                                             skip_group_check=True)
                            for e in range(P):
                                el = ti * P + e
                                nc.tensor.matmul(
                                    psm[:, e:e + 1], we_v[:, :, el],
                                    hsT[:, e:e + 1], start=False,
                                    stop=(e == P - 1), skip_group_check=True)
                            msgT = spool.tile([P, P], f32, name="msgT")
                            if ti % 2 == 0:
                                nc.vector.tensor_copy(msgT[:], psm[:])
                            else:
                                nc.scalar.copy(msgT[:], psm[:])
                            psm2 = psum_m.tile([P, P], f32, name="msg_t_ps",
                                               tag="m")
                            nc.tensor.transpose(psm2[:], msgT[:], idf_sb[:])
                            msgE = spool.tile([P, P], f32, name="msgE")
                            nc.scalar.copy(msgE[:], psm2[:])

                            # scatter matmul + row scatter
                            psa = psum_m.tile([P, P], f32, name="agg_ps", tag="m")
                            nc.tensor.matmul(psa[:], smat_sb[:, t * P:(t + 1) * P],
                                             msgE[:], start=True, stop=True)
                            aggu = spool.tile([P, P], f32, name="aggu")
                            nc.vector.tensor_copy(aggu[:], psa[:])
                            nc.gpsimd.indirect_dma_start(
                                out=aggd[:], in_=aggu[:], in_offset=None,
                                out_offset=IndirectOffsetOnAxis(
                                    ap=rowu_sb[:, t:t + 1], axis=0),
                            )

                    # aggT [128o, NS] via on-chip PE transposes of agg row tiles
                    aggT = spool.tile([H, NS], f32, name="aggT")
                    for j in range(NS // P):
                        agr = gpool.tile([P, H], f32, name="agr")
                        nc.sync.dma_start(agr[:], aggd[j * P:(j + 1) * P, :])
                        psq = psum_m.tile([P, P], f32, name="agr_ps", tag="m")
                        nc.tensor.transpose(psq[:], agr[:], idf_sb[:])
                        nc.scalar.copy(aggT[:, j * P:(j + 1) * P], psq[:])

                    outT = spool.tile([H, NS], f32, name="outT")
                    nc.vector.tensor_add(outT[:], aggT[:], ps_root[:])

                    # BN stats: global sum & sum-of-squares over nodes
                    stats = statp.tile([H, 2], f32, name="stats")
                    nc.vector.tensor_reduce(stats[:, 0:1], outT[:],
                                            axis=mybir.AxisListType.X, op=ALU.add)
                    trash = spool.tile([H, NS], f32, name="trash")
                    nc.scalar.activation(trash[:], outT[:], AXF.Square,
                                         accum_out=stats[:, 1:2])
                    st_in = dramp.tile([H, 2], f32, name="st_in")
                    nc.sync.dma_start(st_in[:], stats[:])
                    st_out = dramp.tile([H, 2], f32, name="st_out",
                                        addr_space="Shared")
                    nc.gpsimd.collective_compute(
                        "AllReduce", ALU.add, replica_groups=groups,
                        ins=[st_in.opt()], outs=[st_out.opt()])
                    stats2 = statp.tile([H, 2], f32, name="stats2")
                    nc.sync.dma_start(stats2[:], st_out[:])

                    mu = statp.tile([H, 1], f32, name="mu")
                    nc.scalar.mul(mu[:], stats2[:, 0:1], 1.0 / N)
                    ex2 = statp.tile([H, 1], f32, name="ex2")
                    nc.scalar.mul(ex2[:], stats2[:, 1:2], 1.0 / N)
                    musq = statp.tile([H, 1], f32, name="musq")
                    nc.vector.tensor_mul(musq[:], mu[:], mu[:])
                    var = statp.tile([H, 1], f32, name="var")
                    nc.vector.tensor_tensor(out=var[:], in0=ex2[:], in1=musq[:],
                                            op=ALU.subtract)
                    std = statp.tile([H, 1], f32, name="std")
                    nc.scalar.activation(std[:], var[:], AXF.Sqrt,
                                         bias=eps_sb[:, 0:1])
                    rstd = statp.tile([H, 1], f32, name="rstd")
                    nc.vector.reciprocal(rstd[:], std[:])
                    scal = statp.tile([H, 1], f32, name="scal")
                    nc.vector.tensor_mul(scal[:], rstd[:], bng_sb[l][:])
                    mscal = statp.tile([H, 1], f32, name="mscal")
                    nc.vector.tensor_mul(mscal[:], mu[:], scal[:])
                    shift = statp.tile([H, 1], f32, name="shift")
                    nc.vector.tensor_tensor(out=shift[:], in0=bnb_sb[l][:],
                                            in1=mscal[:], op=ALU.subtract)

                    relu_o = spool.tile([H, NS], f32, name="relu_o")
                    nc.scalar.activation(relu_o[:], outT[:], AXF.Relu,
                                         bias=shift[:, 0:1], scale=scal[:, 0:1])
                    nc.vector.tensor_add(hT[:], hT[:], relu_o[:])

                    # write updated slice (rows, bf16) + AllGather
                    hsl = dramp.tile([NS, H], bf16, name="hsl")
                    for j in range(NS // P):
                        pst = psum_m.tile([P, P], f32, name="hup_ps", tag="m")
                        nc.tensor.transpose(pst[:], hT[:, j * P:(j + 1) * P],
                                            idf_sb[:])
                        hrow = spool.tile([P, H], bf16, name="hup_row")
                        nc.scalar.copy(hrow[:], pst[:])
                        nc.sync.dma_start(hsl[j * P:(j + 1) * P, :], hrow[:])
                    # NOTE: Local (not Shared) — indirect-DMA gather reads this
                    # tensor, and gathering from the Shared window hangs HW.
                    hfull = dramp.tile([N, H], bf16, name="hfull")
                    nc.gpsimd.collective_compute(
                        "AllGather", ALU.bypass, replica_groups=groups,
                        ins=[hsl.opt()], outs=[hfull.opt()])
                    h_rows = hfull

            # ---- head (all cores redundantly) ----
            with tc.tile_pool(name="headp", bufs=2) as headp:
                pmat_sb = headp.tile([P, 32 * G], bf16, bufs=1)
                nc.sync.dma_start(pmat_sb[:], pmat_d[:])
                hw1_sb = headp.tile([H, H], bf16, bufs=1)
                nc.sync.dma_start(hw1_sb[:], hw1_d[:])
                hb1_sb = headp.tile([H, 1], f32, bufs=1)
                nc.sync.dma_start(hb1_sb[:], hb1_d[:])
                hw2_sb = headp.tile([H, 1], bf16, bufs=1)
                nc.sync.dma_start(hw2_sb[:], hw2_d[:])
                hb2_sb = headp.tile([1, 1], f32, bufs=1)
                nc.sync.dma_start(hb2_sb[:], hb2_d[:])

                ps_pool = psum_root.tile([H, G], f32, name="pool_ps", tag="r")
                for i in range(N // P):
                    hrt = headp.tile([P, H], bf16, name="hd_rows")
                    nc.sync.dma_start(hrt[:], h_rows[i * P:(i + 1) * P, :])
                    nc.tensor.matmul(ps_pool[:], hrt[:],
                                     pmat_sb[:, i * G:(i + 1) * G],
                                     start=(i == 0), stop=(i == N // P - 1))
                pooledT = headp.tile([H, G], bf16, name="pooledT")
                nc.vector.tensor_copy(pooledT[:], ps_pool[:])
                ps_z = psum_m.tile([H, G], f32, name="z_ps", tag="m")
                nc.tensor.matmul(ps_z[:], hw1_sb[:], pooledT[:],
                                 start=True, stop=True)
                z = headp.tile([H, G], bf16, name="z")
                nc.scalar.activation(z[:], ps_z[:], AXF.Relu, bias=hb1_sb[:, 0:1])
                ps_y = psum_m.tile([1, G], f32, name="y_ps", tag="m")
                nc.tensor.matmul(ps_y[:], hw2_sb[:], z[:], start=True, stop=True)
                ysb = headp.tile([1, G], f32, name="ysb")
                nc.vector.tensor_scalar_add(ysb[:], ps_y[:], hb2_sb[0:1, 0:1])
                nc.sync.dma_start(y_d[:], ysb[:])

    nc.compile()
    return nc


# ----------------------------------------------------------------------------
# Entry point
# ----------------------------------------------------------------------------

def kernel(**inputs):
    inp = {k: np.asarray(v) for k, v in inputs.items()}
    cores, EP, T = _preprocess(inp["edge_index"], inp["edge_attr"])

    bf = lambda a: np.ascontiguousarray(np.asarray(a, np.float32)).astype(BF16)
    f32 = lambda a: np.ascontiguousarray(np.asarray(a, np.float32))

    # shared (replicated) tensors
    e1w = np.concatenate(
        [np.asarray(inp["e1_w"], np.float32),
         np.asarray(inp["e1_b"], np.float32)[:, None, :]], axis=1)  # [L,17,128]
    # W2P[l][k, o*128+h] = e2_w[l][k, h*128+o]
    w2p = np.asarray(inp["e2_w"], np.float32).reshape(L, H, H, H) \
        .transpose(0, 1, 3, 2).reshape(L, H, H * H)
    b2 = np.asarray(inp["e2_b"], np.float32).reshape(L, H, H)
    xa = np.concatenate([np.asarray(inp["x"], np.float32).T,
                         np.ones((1, N), np.float32)], 0)  # [65, N]
    nw = np.concatenate([np.asarray(inp["node_w"], np.float32),
                         np.asarray(inp["node_b"], np.float32)[None, :]], 0)

    batch = np.asarray(inp["batch"], np.int64)
    cnt = np.bincount(batch, minlength=G).astype(np.float32)
    Pm = np.zeros((N, G), np.float32)
    Pm[np.arange(N), batch] = 1.0 / np.maximum(cnt, 1.0)[batch]
    pmat = np.zeros((P, 32 * G), np.float32)
    for i in range(32):
        pmat[:, i * G:(i + 1) * G] = Pm[i * P:(i + 1) * P]

    shared = dict(
        e1w=bf(e1w), w2p=bf(w2p), b2=bf(b2),
        rw=bf(inp["root_w"]),
        bng=f32(inp["bn_g"])[:, :, None], bnb=f32(inp["bn_b"])[:, :, None],
        xa=bf(xa), nw=bf(nw),
        pmat=bf(pmat), hw1=bf(inp["head_w1"]),
        hb1=f32(inp["head_b1"])[:, None], hw2=bf(inp["head_w2"]),
        hb2=f32(inp["head_b2"])[None, :],
        idf=np.eye(P, dtype=np.float32), idb=np.eye(P, dtype=np.float32).astype(BF16),
    )

    in_maps = []
    for c in range(NC):
        cd = cores[c]
        m = dict(shared)
        m["ea"] = bf(cd["eaT"])
        m["xs"] = bf(xa[:, c * NS:(c + 1) * NS])
        m["srcg"] = np.ascontiguousarray(cd["srcg"].reshape(T, P).T)
        m["rowu"] = np.ascontiguousarray(cd["rows_u"].T)
        m["smat"] = np.ascontiguousarray(
            cd["S"].transpose(1, 0, 2).reshape(P, T * P))
        in_maps.append(m)

    nc = _build(EP, T)
    import os
    trace = os.environ.get("KERNEL_TRACE", "0") == "1"
    res = run_bass_kernel_spmd(nc, in_maps, list(range(NC)), trace=trace)
    if trace and res.exec_time_ns is not None:
        print(f"HW exec time: {res.exec_time_ns} ns")
    y = np.asarray(res.results[0]["y"], np.float32).reshape(G)
    return y
